# revision 29
# baseline (speedup 1.0000x reference)
"""CenterLoss (gather + MSE mean) on 8 Trainium2 NeuronCores.

Strategy (data-parallel + label-sort, per sharding hint):
  Expand  sum(x-c_l)^2 = sum x^2 + sum_l (n_l |c_l|^2 - 2 S_l . c_l),
  S_l = sum of x rows with label l, and sort rows by label on the host
  (a legal data-parallel resharding: the mean is order-invariant).
  After the global sort each 1024-row chunk touches only ~9 distinct
  classes (<= CH_CLS=32 with huge margin), so the center terms need
  only a 32-row table window per chunk:

  - Shard the sorted x / labels along N across 8 cores.
  - Per core, stream x in [128, 8, 512] f32 chunks (16 chunks of 2MB).
    ACT squares each chunk in place with a row-accumulate -> sum x^2.
  - DVE casts the chunk to bf16; PE computes 2*S per chunk as 8 bf16
    matmuls (lhsT = host-built one-hot A with entries 2.0 — exact in
    bf16) accumulating in one PSUM bank.
  - gpsimd.dma_gather pulls the chunk's <=32 distinct center rows
    (f32, 2KB each, 64KB/chunk) from the DRAM table. Two stock DVE
    scalar_tensor_tensor ops then fold the whole center correction:
      w  = (c * n) - 2S          (counts n as per-partition scalar)
      accum(w * c) = sum_f n|c|^2 - 2 S.c   -> crossc[:, chunk]
  - Epilogue: fold crossc into the x^2 column tile, free-dim reduce,
    ones-matmul partition reduce -> per-core scalar. Host sums the 8
    partials and divides by N*FEAT.

  vs. the direct gather+subtract kernel this removes the 8MB/core
  center-gather HBM traffic (~20% of total) and the 70us DVE subtract;
  HBM traffic is ~33.6MB/core, within ~15% of the streaming floor.

  HW notes baked in below: custom-DVE ops (tensor_tensor_reduce) wedge
  the device on this runtime path — stock ops only. GpSimd tensor ops
  convoy with its own gather dispatches — keep it to gathers. The
  HWDGE engines share one FIFO ring set, so DMA issue order is load
  order: idx, x0, A piece 0, x1, then the rest.
"""
import numpy as np
from contextlib import ExitStack

import ml_dtypes

import concourse.tile as tile
from concourse import bacc, mybir
from concourse.bass_utils import run_bass_kernel_spmd

N, FEAT, NCLASS = 131072, 512, 1000
NCORES = 8
SHARD = N // NCORES          # 16384 rows per core
CHUNK = 1024                 # rows per pipeline chunk
T = SHARD // CHUNK           # 16 chunks
ROWS_P = CHUNK // 128        # 8 rows per partition per chunk
CH_CLS = 32                  # center-window slots per chunk (~9 used)

TRACE = False                # set by test.py for profiled runs
LAST_RESULTS = None          # BassKernelResults of the last kernel() call


def _build_nc():
    nc = bacc.Bacc("TRN2", target_bir_lowering=False, debug=False,
                   enable_asserts=False, num_swdge_queues=4)
    x = nc.dram_tensor("x", [SHARD, FEAT], mybir.dt.float32,
                       kind="ExternalInput")
    a = nc.dram_tensor("a", [128, T * ROWS_P * CH_CLS], mybir.dt.bfloat16,
                       kind="ExternalInput")
    idxs = nc.dram_tensor("idxs", [128, T * CH_CLS // 16], mybir.dt.int16,
                          kind="ExternalInput")
    cnts = nc.dram_tensor("cnts", [CH_CLS, T], mybir.dt.float32,
                          kind="ExternalInput")
    tbl = nc.dram_tensor("tbl", [NCLASS, FEAT], mybir.dt.float32,
                         kind="ExternalInput")
    out = nc.dram_tensor("out", [1, 1], mybir.dt.float32,
                         kind="ExternalOutput")

    with tile.TileContext(nc) as tc, ExitStack() as ctx:
        xp = ctx.enter_context(tc.tile_pool(name="xp", bufs=7))
        xbp = ctx.enter_context(tc.tile_pool(name="xbp", bufs=4))
        cp = ctx.enter_context(tc.tile_pool(name="cp", bufs=4))
        scp = ctx.enter_context(tc.tile_pool(name="scp", bufs=4))
        sp = ctx.enter_context(tc.tile_pool(name="small", bufs=1))
        pp = ctx.enter_context(tc.tile_pool(name="pp", bufs=4, space="PSUM"))

        # DMA issue order matters: the HWDGE engines share one FIFO ring
        # set, so anything queued ahead of x delays the stream. Issue the
        # tiny idx load, the first x chunks, and A piece 0 up front; the
        # remaining pieces follow (piece i is first needed by chunk 4i).
        xr = x.ap().rearrange("(t p u) f -> t p u f", t=T, p=128)
        idxt = sp.tile([128, T * CH_CLS // 16], mybir.dt.int16)
        nc.scalar.dma_start(idxt[:], idxs.ap())
        APC = 4
        acols = T * ROWS_P * CH_CLS // APC
        a_sb = sp.tile([128, T * ROWS_P * CH_CLS], mybir.dt.bfloat16)
        cntt = sp.tile([CH_CLS, T], mybir.dt.float32)
        nc.scalar.dma_start(cntt[:], cnts.ap())
        for i in range(APC):
            nc.scalar.dma_start(a_sb[:, i * acols:(i + 1) * acols],
                                a.ap()[:, i * acols:(i + 1) * acols])

        # acc columns: [0:T] = per-chunk sum x^2 (all 128 rows),
        # col T = sum_l n_l|c_l|^2 - 2 S.c  (rows 0:32).
        acc = sp.tile([128, T + 1], mybir.dt.float32)
        nc.vector.memset(acc[:], 0.0)
        crossc = sp.tile([CH_CLS, T], mybir.dt.float32)

        def center_ops(st, ct, t):
            # Center correction in two fused stock DVE ops (custom-DVE
            # wedges the device; a separate PSUM->SBUF copy re-creates the
            # CAST->PE->COPY->CAST convoy). PSUM reads directly here.
            w = scp.tile([CH_CLS, FEAT], mybir.dt.float32)
            nc.vector.scalar_tensor_tensor(
                out=w[:], in0=ct[0:CH_CLS, 0, :], scalar=cntt[:, t:t + 1],
                in1=st[:], op0=mybir.AluOpType.mult,
                op1=mybir.AluOpType.subtract)
            sc = scp.tile([CH_CLS, FEAT], mybir.dt.float32)
            nc.vector.scalar_tensor_tensor(
                out=sc[:], in0=w[:], scalar=1.0, in1=ct[0:CH_CLS, 0, :],
                op0=mybir.AluOpType.bypass, op1=mybir.AluOpType.mult,
                accum_out=crossc[:, t:t + 1])

        iw = CH_CLS // 16            # idx columns per chunk
        pend = None                  # (st, ct, t) awaiting center ops
        for t in range(T):
            xt = xp.tile([128, ROWS_P, FEAT], mybir.dt.float32)
            nc.sync.dma_start(xt[:], xr[t])
            ct = cp.tile([128, 1, FEAT], mybir.dt.float32)
            nc.gpsimd.dma_gather(ct[:], tbl.ap(),
                                 idxt[:, t * iw:(t + 1) * iw],
                                 CH_CLS, CH_CLS, FEAT, queue_num=t % 4)
            xb = xbp.tile([128, ROWS_P, FEAT], mybir.dt.bfloat16)
            nc.vector.tensor_copy(xb[:], xt[:])
            st = pp.tile([CH_CLS, FEAT], mybir.dt.float32, space="PSUM")
            for u in range(ROWS_P):
                col = (t * ROWS_P + u) * CH_CLS
                nc.tensor.matmul(st[:],
                                 lhsT=a_sb[:, col:col + CH_CLS],
                                 rhs=xb[:, u, :],
                                 start=(u == 0), stop=(u == ROWS_P - 1))
            # sum x^2 of the chunk (in place, after the cast's read).
            nc.scalar.activation(xt[:], xt[:],
                                 mybir.ActivationFunctionType.Square,
                                 accum_out=acc[:, t:t + 1])
            # Chunk t's center ops are emitted in iteration t+1 so the DVE
            # FIFO never makes CAST(t+1) wait behind a PE-dependent op —
            # that cross-engine cycle (CAST -> PE -> COPY -> next CAST)
            # throttled the whole pipeline to ~7.3us/chunk.
            if pend is not None:
                center_ops(*pend)
            pend = (st, ct, t)
        center_ops(*pend)

        # Epilogue: fold the per-chunk center corrections into acc's last
        # column, then one global reduce and a ones-matmul partition sum.
        nc.vector.tensor_reduce(acc[0:CH_CLS, T:T + 1], crossc[:],
                                mybir.AxisListType.X, mybir.AluOpType.add)
        ones = sp.tile([128, 1], mybir.dt.float32)
        nc.vector.memset(ones[:], 1.0)
        red = sp.tile([128, 1], mybir.dt.float32)
        nc.vector.tensor_reduce(red[:], acc[:], mybir.AxisListType.X,
                                mybir.AluOpType.add)
        tot = pp.tile([1, 1], mybir.dt.float32, space="PSUM")
        nc.tensor.matmul(tot[:], lhsT=red[:], rhs=ones[:],
                         start=True, stop=True)
        tot_sb = sp.tile([1, 1], mybir.dt.float32)
        nc.vector.tensor_copy(tot_sb[:], tot[:])
        nc.sync.dma_start(out.ap(), tot_sb[:])
    nc.compile()
    return nc


_NC = None


def _get_nc():
    global _NC
    if _NC is None:
        _NC = _build_nc()
    return _NC


def _prep_core(labels_shard):
    """Per-core host prep from the SORTED label shard: one-hot A tiles
    (entries 2.0 so PE emits 2S), wrapped int16 gather indices, and
    per-(slot, chunk) counts."""
    A = np.zeros((128, T * ROWS_P * CH_CLS), dtype=ml_dtypes.bfloat16)
    idx16 = np.zeros((16, T * CH_CLS // 16), dtype=np.int16)
    counts = np.zeros((CH_CLS, T), dtype=np.float32)
    p_idx = np.arange(CHUNK) // ROWS_P
    u_idx = np.arange(CHUNK) % ROWS_P
    iw = CH_CLS // 16
    for t in range(T):
        lab = labels_shard[t * CHUNK:(t + 1) * CHUNK]
        classes, cnt = np.unique(lab, return_counts=True)
        k = len(classes)
        assert k <= CH_CLS, f"chunk spans {k} classes > {CH_CLS}"
        win = np.full(CH_CLS, classes[-1], dtype=np.int64)
        win[:k] = classes
        counts[:k, t] = cnt
        slot = np.searchsorted(win[:k], lab)
        A3 = np.zeros((128, ROWS_P, CH_CLS), dtype=np.float32)
        A3[p_idx, u_idx, slot] = 2.0
        A[:, t * ROWS_P * CH_CLS:(t + 1) * ROWS_P * CH_CLS] = \
            A3.reshape(128, ROWS_P * CH_CLS)
        idx16[:, t * iw:(t + 1) * iw] = \
            win.reshape(iw, 16).T.astype(np.int16)
    return A, np.tile(idx16, (8, 1)), counts


def kernel(input_x, input_labels, target_x):
    global LAST_RESULTS
    input_x = np.ascontiguousarray(np.asarray(input_x), dtype=np.float32)
    labels = np.asarray(input_labels).astype(np.int64)
    table = np.ascontiguousarray(np.asarray(target_x), dtype=np.float32)
    assert input_x.shape == (N, FEAT) and labels.shape == (N,)
    assert table.shape == (NCLASS, FEAT)

    perm = np.argsort(labels, kind="stable")
    xs = input_x[perm]
    ls = labels[perm]

    nc = _get_nc()
    in_maps = []
    for c in range(NCORES):
        sl = slice(c * SHARD, (c + 1) * SHARD)
        A, idx16, counts = _prep_core(ls[sl])
        in_maps.append({
            "x": xs[sl],
            "a": A,
            "idxs": idx16,
            "cnts": counts,
            "tbl": table,
        })
    res = run_bass_kernel_spmd(nc, in_maps, list(range(NCORES)), trace=TRACE)
    LAST_RESULTS = res
    partials = [np.float64(r["out"][0, 0]) for r in res.results]
    return np.float32(sum(partials) / (N * FEAT))


# revision 31
# speedup vs baseline: 1.2040x; 1.2040x over previous
"""CenterLoss (gather + MSE mean) on 8 Trainium2 NeuronCores.

Strategy (data-parallel + label-sort, per sharding hint):
  Expand  sum(x-c_l)^2 = sum x^2 + sum_l (n_l |c_l|^2 - 2 S_l . c_l),
  S_l = sum of x rows with label l, and sort rows by label on the host
  (a legal data-parallel resharding: the mean is order-invariant).
  After the global sort each 1024-row chunk touches only ~9 distinct
  classes (<= CH_CLS=32 with huge margin), so the center terms need
  only a 32-row table window per chunk:

  - Shard the sorted x / labels along N across 8 cores.
  - Per core, stream x in [128, 8, 512] f32 chunks (16 chunks of 2MB).
    ACT squares each chunk in place with a row-accumulate -> sum x^2.
  - DVE casts the chunk to bf16; PE computes 2*S per chunk as 8 bf16
    matmuls (lhsT = host-built one-hot A with entries 2.0 — exact in
    bf16). Four consecutive chunks stack into ONE PSUM bank at
    partition offsets 0/32/64/96 (the out AP's base partition routes
    the PE tile), so the center correction runs once per 4-chunk
    group: one 128-row dma_gather of the table windows and two fused
    stock DVE ops
      w = (c * n) - 2S ;  accum(w * c) -> crossc[:, group]
    which equals sum_f n|c|^2 - 2 S.c per window row.
  - Epilogue: fold crossc into the x^2 column tile, free-dim reduce,
    ones-matmul partition reduce -> per-core scalar. Host sums the 8
    partials and divides by N*FEAT.

  vs. the direct gather+subtract kernel this removes the 8MB/core
  center-gather HBM traffic (~20% of total) and the 70us DVE subtract;
  HBM traffic is ~33.6MB/core, within ~15% of the streaming floor.

  HW notes baked in below: custom-DVE ops (tensor_tensor_reduce) wedge
  the device on this runtime path — stock ops only. GpSimd tensor ops
  convoy with its own gather dispatches — keep it to gathers. Per-chunk
  DVE window ops re-create a CAST->PE->STT->CAST convoy (~7us/chunk);
  batching them per 4 chunks breaks the cycle.
"""
import numpy as np
from contextlib import ExitStack

import ml_dtypes

import concourse.tile as tile
from concourse import bacc, mybir
from concourse.bass_utils import run_bass_kernel_spmd

N, FEAT, NCLASS = 131072, 512, 1000
NCORES = 8
SHARD = N // NCORES          # 16384 rows per core
CHUNK = 1024                 # rows per pipeline chunk
T = SHARD // CHUNK           # 16 chunks
ROWS_P = CHUNK // 128        # 8 rows per partition per chunk
CH_CLS = 32                  # center-window slots per chunk (~9 used)
GRP = 4                      # chunks per PSUM bank / center-ops group
G = T // GRP                 # groups

TRACE = False                # set by test.py for profiled runs
LAST_RESULTS = None          # BassKernelResults of the last kernel() call


def _build_nc():
    nc = bacc.Bacc("TRN2", target_bir_lowering=False, debug=False,
                   enable_asserts=False, num_swdge_queues=4)
    x = nc.dram_tensor("x", [SHARD, FEAT], mybir.dt.float32,
                       kind="ExternalInput")
    a = nc.dram_tensor("a", [128, T * ROWS_P * CH_CLS], mybir.dt.bfloat16,
                       kind="ExternalInput")
    idxs = nc.dram_tensor("idxs", [128, G * GRP * CH_CLS // 16],
                          mybir.dt.int16, kind="ExternalInput")
    cnts = nc.dram_tensor("cnts", [GRP * CH_CLS, G], mybir.dt.float32,
                          kind="ExternalInput")
    tbl = nc.dram_tensor("tbl", [NCLASS, FEAT], mybir.dt.float32,
                         kind="ExternalInput")
    out = nc.dram_tensor("out", [1, 1], mybir.dt.float32,
                         kind="ExternalOutput")

    with tile.TileContext(nc) as tc, ExitStack() as ctx:
        xp = ctx.enter_context(tc.tile_pool(name="xp", bufs=7))
        xbp = ctx.enter_context(tc.tile_pool(name="xbp", bufs=4))
        cp = ctx.enter_context(tc.tile_pool(name="cp", bufs=2))
        scp = ctx.enter_context(tc.tile_pool(name="scp", bufs=2))
        sp = ctx.enter_context(tc.tile_pool(name="small", bufs=1))
        pp = ctx.enter_context(tc.tile_pool(name="pp", bufs=2, space="PSUM"))

        xr = x.ap().rearrange("(t p u) f -> t p u f", t=T, p=128)
        idxt = sp.tile([128, G * GRP * CH_CLS // 16], mybir.dt.int16)
        nc.scalar.dma_start(idxt[:], idxs.ap())
        cntt = sp.tile([GRP * CH_CLS, G], mybir.dt.float32)
        nc.scalar.dma_start(cntt[:], cnts.ap())
        APC = 4
        acols = T * ROWS_P * CH_CLS // APC
        a_sb = sp.tile([128, T * ROWS_P * CH_CLS], mybir.dt.bfloat16)
        for i in range(APC):
            nc.scalar.dma_start(a_sb[:, i * acols:(i + 1) * acols],
                                a.ap()[:, i * acols:(i + 1) * acols])

        # acc columns: [0:T] = per-chunk sum x^2, col T = center corr.
        acc = sp.tile([128, T + 1], mybir.dt.float32)
        nc.vector.memset(acc[:], 0.0)
        crossc = sp.tile([128, G], mybir.dt.float32)

        def center_ops(st, ct, g):
            # One pass per 4-chunk group, two fused stock DVE ops
            # (custom-DVE wedges the device), PSUM read directly.
            w = scp.tile([GRP * CH_CLS, FEAT], mybir.dt.float32)
            nc.vector.scalar_tensor_tensor(
                out=w[:], in0=ct[:, 0, :], scalar=cntt[:, g:g + 1],
                in1=st[:], op0=mybir.AluOpType.mult,
                op1=mybir.AluOpType.subtract)
            sc = scp.tile([GRP * CH_CLS, FEAT], mybir.dt.float32)
            nc.vector.scalar_tensor_tensor(
                out=sc[:], in0=w[:], scalar=1.0, in1=ct[:, 0, :],
                op0=mybir.AluOpType.bypass, op1=mybir.AluOpType.mult,
                accum_out=crossc[:, g:g + 1])

        giw = GRP * CH_CLS // 16     # idx columns per group
        pend = None                  # (st, ct, g) awaiting center ops
        st = ct = None
        for t in range(T):
            g, j = divmod(t, GRP)
            xt = xp.tile([128, ROWS_P, FEAT], mybir.dt.float32)
            nc.sync.dma_start(xt[:], xr[t])
            if j == 0:
                ct = cp.tile([128, 1, FEAT], mybir.dt.float32)
                nc.gpsimd.dma_gather(ct[:], tbl.ap(),
                                     idxt[:, g * giw:(g + 1) * giw],
                                     GRP * CH_CLS, GRP * CH_CLS, FEAT,
                                     queue_num=g % 4)
                st = pp.tile([GRP * CH_CLS, FEAT], mybir.dt.float32,
                             space="PSUM")
            xb = xbp.tile([128, ROWS_P, FEAT], mybir.dt.bfloat16)
            nc.vector.tensor_copy(xb[:], xt[:])
            for u in range(ROWS_P):
                col = (t * ROWS_P + u) * CH_CLS
                nc.tensor.matmul(st[j * CH_CLS:(j + 1) * CH_CLS, :],
                                 lhsT=a_sb[:, col:col + CH_CLS],
                                 rhs=xb[:, u, :],
                                 start=(u == 0), stop=(u == ROWS_P - 1),
                                 tile_position=(0, j * CH_CLS))
            # sum x^2 of the chunk (in place, after the cast's read).
            nc.scalar.activation(xt[:], xt[:],
                                 mybir.ActivationFunctionType.Square,
                                 accum_out=acc[:, t:t + 1])
            if j == GRP - 1:
                if pend is not None:
                    center_ops(*pend)
                pend = (st, ct, g)
        center_ops(*pend)

        # Epilogue: fold the center corrections into acc's last column,
        # then one global reduce and a ones-matmul partition sum.
        nc.vector.tensor_reduce(acc[:, T:T + 1], crossc[:],
                                mybir.AxisListType.X, mybir.AluOpType.add)
        ones = sp.tile([128, 1], mybir.dt.float32)
        nc.vector.memset(ones[:], 1.0)
        red = sp.tile([128, 1], mybir.dt.float32)
        nc.vector.tensor_reduce(red[:], acc[:], mybir.AxisListType.X,
                                mybir.AluOpType.add)
        tot = pp.tile([1, 1], mybir.dt.float32, space="PSUM")
        nc.tensor.matmul(tot[:], lhsT=red[:], rhs=ones[:],
                         start=True, stop=True)
        tot_sb = sp.tile([1, 1], mybir.dt.float32)
        nc.vector.tensor_copy(tot_sb[:], tot[:])
        nc.sync.dma_start(out.ap(), tot_sb[:])
    nc.compile()
    return nc


_NC = None


def _get_nc():
    global _NC
    if _NC is None:
        _NC = _build_nc()
    return _NC


def _prep_core(labels_shard):
    """Per-core host prep from the SORTED label shard: one-hot A tiles
    (entries 2.0 so PE emits 2S), wrapped int16 gather indices per
    4-chunk group, and per-(group-slot, group) counts."""
    A = np.zeros((128, T * ROWS_P * CH_CLS), dtype=ml_dtypes.bfloat16)
    idx16 = np.zeros((16, G * GRP * CH_CLS // 16), dtype=np.int16)
    counts = np.zeros((GRP * CH_CLS, G), dtype=np.float32)
    p_idx = np.arange(CHUNK) // ROWS_P
    u_idx = np.arange(CHUNK) % ROWS_P
    giw = GRP * CH_CLS // 16
    for t in range(T):
        g, j = divmod(t, GRP)
        lab = labels_shard[t * CHUNK:(t + 1) * CHUNK]
        classes, cnt = np.unique(lab, return_counts=True)
        k = len(classes)
        assert k <= CH_CLS, f"chunk spans {k} classes > {CH_CLS}"
        win = np.full(CH_CLS, classes[-1], dtype=np.int64)
        win[:k] = classes
        counts[j * CH_CLS:j * CH_CLS + k, g] = cnt
        slot = np.searchsorted(win[:k], lab)
        A3 = np.zeros((128, ROWS_P, CH_CLS), dtype=np.float32)
        A3[p_idx, u_idx, slot] = 2.0
        A[:, t * ROWS_P * CH_CLS:(t + 1) * ROWS_P * CH_CLS] = \
            A3.reshape(128, ROWS_P * CH_CLS)
        # group idx list: window rows of chunk j land at partitions
        # j*32..j*32+31 of the gather output (idx i -> partition i%...,
        # wrapped-16 layout over the 128 group indices).
        base = g * giw
        for i in range(CH_CLS):
            gi = j * CH_CLS + i      # global index within group
            idx16[gi % 16, base + gi // 16] = win[i]
    return A, np.tile(idx16, (8, 1)), counts


def kernel(input_x, input_labels, target_x):
    global LAST_RESULTS
    input_x = np.ascontiguousarray(np.asarray(input_x), dtype=np.float32)
    labels = np.asarray(input_labels).astype(np.int64)
    table = np.ascontiguousarray(np.asarray(target_x), dtype=np.float32)
    assert input_x.shape == (N, FEAT) and labels.shape == (N,)
    assert table.shape == (NCLASS, FEAT)

    perm = np.argsort(labels, kind="stable")
    xs = input_x[perm]
    ls = labels[perm]

    nc = _get_nc()
    in_maps = []
    for c in range(NCORES):
        sl = slice(c * SHARD, (c + 1) * SHARD)
        A, idx16, counts = _prep_core(ls[sl])
        in_maps.append({
            "x": xs[sl],
            "a": A,
            "idxs": idx16,
            "cnts": counts,
            "tbl": table,
        })
    res = run_bass_kernel_spmd(nc, in_maps, list(range(NCORES)), trace=TRACE)
    LAST_RESULTS = res
    partials = [np.float64(r["out"][0, 0]) for r in res.results]
    return np.float32(sum(partials) / (N * FEAT))
